# revision 17
# baseline (speedup 1.0000x reference)
"""DA-RNN (dual-stage attention RNN) forward pass on 8 TRN2 NeuronCores.

Data-parallel: batch 2048 sharded 256 per core, weights replicated.

Algebraic structure exploited (validated against the reference in numpy):
  * Both attention blocks add their state-dependent term as a per-sample
    constant across the softmax axis, so softmax cancels it.  The encoder
    input attention (a1, a2) and the decoder temporal attention (beta) are
    input-only precomputes, and the decoder context vector is constant
    across decoder steps.
  * context only enters through dot products (fc_W, fc_final_W): three
    matvec columns [v, fcW1, w_c] against X_encoded give score/q/r per
    (sample, t); softmax-weighted sums of q, r give the decoder LSTM input
    offset and the output contribution.
  * The decoder LSTM input is scalar per sample: K=2 augmented matmul
    (value row + ones row carrying the bias), row-tiled 4x concurrent.

Precision plan (validated numerically on the fixed inputs, HW 7.2e-3):
  * All hidden (recurrent) weights are fp8e4 (halves weight DMA);
    states stay fp16 and stream as the matmul moving operand (mixed
    fp8-weight x fp16-state matmuls run at full bf16 rate and, unlike
    fp8-rhs DoubleRow, register as activity to the HAM PE clock gate,
    keeping the array at 2.4 GHz).  fp8 state quantization was measured
    to break the error budget; fp8 weights alone cost ~7e-3.
  * Input matmuls (K=17/16/2) are packed 4x concurrent via row-tiled
    tile_position strips with replicated weights/rhs.
  * Within a PSUM bank, each accumulation group's chain completes before
    the next group's start=True (a start clears has_written bank-wide).

On-chip layout: feature-major [dim on partitions (128-chunks), batch on
free].  LSTM states stay in that layout so no transposes in the
recurrences.  PSUM accumulates fp32 throughout.
"""

import sys
import os

sys.path.insert(0, "/opt/trn_rl_repo")
os.environ.setdefault("MYCRO_LOCAL_CACHE", "1")

from contextlib import ExitStack

import numpy as np
import ml_dtypes

import concourse.bass as bass
import concourse.mybir as mybir
import concourse.tile as tile
from concourse import bacc
from concourse.bass_utils import run_bass_kernel_spmd
from concourse.masks import make_identity

F32 = mybir.dt.float32
F16 = mybir.dt.float16
F8 = mybir.dt.float8e4
AF = mybir.ActivationFunctionType
ALU = mybir.AluOpType
DR = mybir.MatmulPerfMode.DoubleRow

NCORES = 8
P = 128
BS = 256          # batch per core
NJ = 2            # 128-partition batch chunks
T = 9             # recurrence steps (T-1 in the reference)
H = 512
IN2 = 15
ME = 16           # encoder gate chunks (4H/128)
KE = 4            # encoder hidden chunks (H/128)
MD = 32           # decoder gate chunks (8H/128)
KD = 8            # decoder hidden chunks (2H/128)

S = 1.0           # no state scaling: states stay fp16
INV_S = 1.0
N_WARM_PREFIX = 36
N_WARM_MID = 20


def _np(a):
    return np.asarray(a, dtype=np.float32)


def _f16(a):
    return np.ascontiguousarray(np.asarray(a, dtype=np.float32).astype(np.float16))


def _f8(a):
    return np.ascontiguousarray(
        np.asarray(a, dtype=np.float32).astype(ml_dtypes.float8_e4m3))


def _pack_weights(inp):
    """Host-side weight folding (weight-only transforms; no input math)."""
    Wih1, Whh1 = _np(inp["enc_lstm_Wih"]), _np(inp["enc_lstm_Whh"])
    b1 = _np(inp["enc_lstm_bih"]) + _np(inp["enc_lstm_bhh"])
    Wih2, Whh2 = _np(inp["enc_lstm1_Wih"]), _np(inp["enc_lstm1_Whh"])
    b2 = _np(inp["enc_lstm1_bih"]) + _np(inp["enc_lstm1_bhh"])
    Wd_ih, Wd_hh = _np(inp["dec_lstm_Wih"]), _np(inp["dec_lstm_Whh"])
    bd = _np(inp["dec_lstm_bih"]) + _np(inp["dec_lstm_bhh"])
    attn1_W = _np(inp["dec_attn1_W"])
    attn2_w = _np(inp["dec_attn2_W"])[0]
    fc_W = _np(inp["fc_W"])[0]
    fcf_W = _np(inp["fc_final_W"])[0]

    W1x = attn1_W[:, 4 * H:]                        # (512, 1024)
    v = W1x.T @ attn2_w                             # (1024,)
    fcW1 = fc_W[:2 * H]
    w_c = fcf_W[2 * H:]
    w_d = fcf_W[:2 * H]

    # input+bias weights, replicated at 4 row strips for row-tiled quads
    wia1 = np.concatenate([Wih1.T, b1[None, :]], axis=0)            # (17, 2048)
    wia2 = np.concatenate([Wih2.T, b2[None, :]], axis=0)            # (16, 2048)
    wid = np.stack([Wd_ih[:, 0],
                    bd + Wd_ih[:, 0] * float(_np(inp["fc_b"])[0])],
                   axis=0)                                          # (2, 4096)

    weights = {
        "wia1": _f16(np.broadcast_to(wia1, (4, 17, 4 * H))),
        "wia2": _f16(np.broadcast_to(wia2, (4, 16, 4 * H))),
        "wid": _f16(np.broadcast_to(wid, (4, 2, 8 * H))),
        "whh1": _f8(Whh1.T),                                        # (512, 2048)
        "whh2": _f8(Whh2.T),                                        # (512, 2048)
        "whhd": _f8(Wd_hh.T),                                       # (1024, 4096)
        "v3": _f16(np.stack([v, fcW1, w_c], axis=1)),               # (1024, 3)
        "wd": _f16(w_d[:, None]),                                   # (1024, 1)
    }
    scalars = {
        "Wf": [float(x) for x in _np(inp["enc_attn_W"])[0, 2 * H:]],    # 9 floats
        "w_y": float(fc_W[2 * H]),
        "fcf_b": float(_np(inp["fc_final_b"])[0]),
    }
    return weights, scalars


def _build(scal, upto="full"):
    nc = bacc.Bacc()

    xd = nc.declare_dram_parameter("x", [BS, T, IN2], F32, isOutput=False)
    yd = nc.declare_dram_parameter("y", [BS, T], F32, isOutput=False)
    wia1d = nc.declare_dram_parameter("wia1", [4, 17, 4 * H], F16, isOutput=False)
    wia2d = nc.declare_dram_parameter("wia2", [4, 16, 4 * H], F16, isOutput=False)
    widd = nc.declare_dram_parameter("wid", [4, 2, 8 * H], F16, isOutput=False)
    whh1d = nc.declare_dram_parameter("whh1", [H, 4 * H], F8, isOutput=False)
    whh2d = nc.declare_dram_parameter("whh2", [H, 4 * H], F8, isOutput=False)
    whhdd = nc.declare_dram_parameter("whhd", [2 * H, 8 * H], F8, isOutput=False)
    v3d = nc.declare_dram_parameter("v3", [2 * H, 3], F16, isOutput=False)
    wdd = nc.declare_dram_parameter("wd", [2 * H, 1], F16, isOutput=False)
    outd = nc.declare_dram_parameter("out", [BS, 1], F32, isOutput=True)
    dbgd = (nc.declare_dram_parameter("dbg", [BS, T, 3], F32, isOutput=True)
            if upto == "enc" else None)
    DBG_T1 = os.environ.get("DBG_T1", "")
    dbg2d = (nc.declare_dram_parameter("dbg2", [P, 4, 2, BS], F32, isOutput=True)
             if DBG_T1 else None)

    Wf = scal["Wf"]

    with ExitStack() as ctx:
        tc = ctx.enter_context(tile.TileContext(nc))
        # persistent pools
        pw = ctx.enter_context(tc.tile_pool(name="pw", bufs=1))
        psm = ctx.enter_context(tc.tile_pool(name="psm", bufs=4))     # small f32 scratch
        pu = ctx.enter_context(tc.tile_pool(name="pu", bufs=4))       # cell temp
        pya = ctx.enter_context(tc.tile_pool(name="pya", bufs=1))
        psum_g = ctx.enter_context(tc.tile_pool(name="psum_g", bufs=5, space="PSUM"))
        psum_t = ctx.enter_context(tc.tile_pool(name="psum_t", bufs=1, space="PSUM"))
        psum_q = ctx.enter_context(tc.tile_pool(name="psum_q", bufs=2, space="PSUM"))

        # ---------------- input DMAs + encoder weights (critical path) -----
        yb = pw.tile([P, NJ, T], F32)
        nc.sync.dma_start(out=yb, in_=yd.rearrange("(j p) t -> p j t", p=P))
        xb0 = pw.tile([P, NJ, T, IN2], F32)
        xd_r = xd.rearrange("(j p) t f -> p j t f", p=P)
        for j in range(NJ):
            nc.sync.dma_start(out=xb0[:, j, :, :], in_=xd_r[:, j, :, :])
        wiaS1 = pw.tile([P, 4 * H], F16)
        wiaS2 = pw.tile([P, 4 * H], F16)
        for g in range(4):
            nc.sync.dma_start(out=wiaS1[32 * g:32 * g + 17, :], in_=wia1d[g, :, :])
            nc.sync.dma_start(out=wiaS2[32 * g:32 * g + 16, :], in_=wia2d[g, :, :])
        whh1 = pw.tile([P, KE, 4 * H], F8)
        nc.sync.dma_start(out=whh1, in_=whh1d.rearrange("(k p) m -> p k m", p=P))
        whh2 = pw.tile([P, KE, 4 * H], F8)
        nc.sync.dma_start(out=whh2, in_=whh2d.rearrange("(k p) m -> p k m", p=P))
        v3t = pw.tile([P, KD, 3], F16)
        nc.sync.dma_start(out=v3t, in_=v3d.rearrange("(k p) c -> p k c", p=P))
        wdt = pw.tile([P, KD, 1], F16)
        nc.sync.dma_start(out=wdt, in_=wdd.rearrange("(k p) c -> p k c", p=P))

        ident = pw.tile([P, P], F16)
        make_identity(nc, ident)

        # PE warm-up: fp16 dummy matmuls so the HAM clock gate reaches 8/8
        # before the encoder starts.
        def _warm(n):
            for _ in range(n):
                wps = psum_q.tile([P, P], F32, name="wps", tag="psq")
                nc.tensor.matmul(wps, ident, ident, start=True, stop=True)

        _warm(N_WARM_PREFIX)

        # ---------------- decoder weights (overlap with encoder) ----------
        whhd = pw.tile([P, KD, 8 * H], F8)
        nc.gpsimd.dma_start(out=whhd, in_=whhdd.rearrange("(k p) m -> p k m", p=P))
        widR = pw.tile([P, 8 * H], F16)
        for g in range(4):
            nc.gpsimd.dma_start(out=widR[32 * g:32 * g + 2, :], in_=widd[g, :, :])

        # persistent state / small tiles
        sqr = pw.tile([P, NJ, T, 3], F32)
        A_t = pw.tile([P, NJ], F32)
        ctxw = pw.tile([P, NJ], F32)
        ytld2 = pw.tile([P, NJ, 2 * T], F16)
        ytldT = pw.tile([2 * T, BS], F16)
        cd = pw.tile([P, KD, BS], F32)
        osb = pw.tile([P, NJ, 1], F32)
        ytA = [pya.tile([P, BS], F16, name=f"ytA{t}", tag=f"ytA{t}")
               for t in range(T)]

        with tc.tile_pool(name="penc", bufs=1) as penc:
            c1 = penc.tile([P, KE, BS], F32)
            c2 = penc.tile([P, KE, BS], F32)
            xtA1 = [penc.tile([P, BS], F16, name=f"xa1_{t}", tag=f"xa1_{t}")
                    for t in range(T)]
            xtA2 = [penc.tile([P, BS], F16, name=f"xa2_{t}", tag=f"xa2_{t}")
                    for t in range(T)]

            with tc.tile_pool(name="ptmp", bufs=1) as ptmp:
                # ---------------- encoder attention precompute ------------
                xyb = ptmp.tile([P, NJ, T, 16], F32)
                nc.vector.tensor_copy(out=xyb[:, :, :, 0:IN2], in_=xb0)
                nc.vector.tensor_copy(out=xyb[:, :, :, IN2], in_=yb[:, :, :])
                mmb = ptmp.tile([P, NJ, T, IN2], F32)
                nc.vector.tensor_mul(
                    out=mmb,
                    in0=xyb[:, :, :, 0:IN2],
                    in1=yb.unsqueeze(3).to_broadcast([P, NJ, T, IN2]),
                )
                # Wf-weighted sums over t in 3 independent sub-chains per
                # base so the DVE pipeline isn't one long dependency chain.
                bases = []
                for bi, src_ in ((0, xyb), (1, mmb)):
                    Fb = 16 if bi == 0 else IN2
                    parts = []
                    for c in range(3):
                        pb = ptmp.tile([P, NJ, Fb], F32, name=f"b{bi}p{c}")
                        nc.vector.tensor_scalar_mul(out=pb,
                                                    in0=src_[:, :, 3 * c, :],
                                                    scalar1=Wf[3 * c])
                        for t in (3 * c + 1, 3 * c + 2):
                            nc.vector.scalar_tensor_tensor(
                                out=pb, in0=src_[:, :, t, :], scalar=Wf[t],
                                in1=pb, op0=ALU.mult, op1=ALU.add)
                        parts.append(pb)
                    nc.vector.tensor_add(out=parts[0], in0=parts[0], in1=parts[1])
                    nc.vector.tensor_add(out=parts[0], in0=parts[0], in1=parts[2])
                    bases.append(parts[0])
                base1, base2 = bases

                a1 = ptmp.tile([P, NJ, 16], F32)
                a2 = ptmp.tile([P, NJ, IN2], F32)
                for bi, (base, a) in enumerate(((base1, a1), (base2, a2))):
                    for j in range(NJ):
                        ssum = psm.tile([P, 1], F32, name="ssum",
                                        tag=f"ssum{bi}{j}")
                        nc.scalar.activation(out=a[:, j, :], in_=base[:, j, :],
                                             func=AF.Exp, accum_out=ssum)
                        inv = psm.tile([P, 1], F32, name="inv", tag=f"inv{bi}{j}")
                        nc.vector.reciprocal(out=inv, in_=ssum)
                        nc.vector.tensor_scalar_mul(out=a[:, j, :], in0=a[:, j, :],
                                                    scalar1=inv)

                # last column = 1.0 so the transpose yields the ones row that
                # carries the bias through the K-augmented matmul
                xt1 = ptmp.tile([P, NJ, T, 17], F16)
                nc.vector.memset(xt1[:, :, :, 16:17], 1.0)
                nc.vector.tensor_mul(
                    out=xt1[:, :, :, 0:16], in0=xyb,
                    in1=a1.unsqueeze(2).to_broadcast([P, NJ, T, 16]))
                xt2 = ptmp.tile([P, NJ, T, 16], F16)
                nc.vector.memset(xt2[:, :, :, IN2:16], 1.0)
                nc.vector.tensor_mul(
                    out=xt2[:, :, :, 0:IN2], in0=mmb,
                    in1=a2.unsqueeze(2).to_broadcast([P, NJ, T, IN2]))

                for t in range(T):
                    for j in range(NJ):
                        tp1 = psum_g.tile([17, P], F16, name="tp1", tag="psg")
                        nc.tensor.transpose(tp1, xt1[:, j, t, :], ident)
                        nc.scalar.copy(out=xtA1[t][0:17, j * P:(j + 1) * P], in_=tp1)
                        tp2 = psum_g.tile([16, P], F16, name="tp2", tag="psg")
                        nc.tensor.transpose(tp2, xt2[:, j, t, :], ident)
                        nc.vector.tensor_copy(out=xtA2[t][0:16, j * P:(j + 1) * P],
                                              in_=tp2)
                    # replicas at row strips 32/64/96 for 4x row-tiled input MMs
                    for g in range(1, 4):
                        nc.sync.dma_start(out=xtA1[t][32 * g:32 * g + 17, :],
                                          in_=xtA1[t][0:17, :])
                        nc.sync.dma_start(out=xtA2[t][32 * g:32 * g + 16, :],
                                          in_=xtA2[t][0:16, :])
                    if t < 2:
                        _warm(4)

                if upto == "pre":
                    nc.vector.tensor_copy(out=osb, in_=xt1[:, :, 0, 0:1])

            # ---------------- encoder recurrence + score matvecs ----------
            # States in hidden-chunk pairs [P, 2, BS]: exactly the DoubleRow
            # rhs layout.  xe16 = 64*h fp16 (scores), xe8 = fp8 cast (DR rhs).
            with tc.tile_pool(name="px16", bufs=3) as px16, \
                 tc.tile_pool(name="pg", bufs=5) as pg:
                prev16 = None
                for t in range(T if upto != "pre" else 0):
                    xe16 = [px16.tile([P, 2, BS], F16, name=f"xe16_{i}",
                                      tag=f"xe16_{i}") for i in range(4)]
                    for br, (wiaS, nk, whhX, cbr) in enumerate((
                            (wiaS1, 17, whh1, c1),
                            (wiaS2, 16, whh2, c2))):
                        xtA = xtA1[t] if br == 0 else xtA2[t]
                        for kp in range(KE // 2):
                            pss = [psum_g.tile([P, 2, BS], F32, name=f"ps{g}",
                                               tag="psg") for g in range(4)]
                            # Per half: 4x concurrent row-tiled input
                            # matmuls, then the DoubleRow accumulates.  Each
                            # half's chain completes before the next half's
                            # start=True (a start clears has_written for the
                            # whole bank, which would turn a later accumulate
                            # into an overwrite).
                            for half in range(2):
                                for g in range(4):
                                    m = g * KE + 2 * kp + half
                                    nc.tensor.matmul(
                                        pss[g][:, half, :],
                                        wiaS[32 * g:32 * g + nk,
                                             m * P:(m + 1) * P],
                                        xtA[32 * g:32 * g + nk, :],
                                        start=True, stop=(t == 0),
                                        tile_position=(32 * g, 0))
                                if t > 0:
                                    for g in range(4):
                                        m = g * KE + 2 * kp + half
                                        for k in range(KE):
                                            nc.tensor.matmul(
                                                pss[g][:, half, :],
                                                whhX[:, k, m * P:(m + 1) * P],
                                                prev16[2 * br + k // 2][:, k % 2, :],
                                                start=False, stop=(k == KE - 1))
                            if DBG_T1 and t == 1 and br == 0 and kp == 0:
                                dbg2b = pw.tile([P, 4, 2, BS], F32, name="dbg2b")
                                for g in range(4):
                                    nc.vector.tensor_copy(out=dbg2b[:, g, :, :],
                                                          in_=pss[g])
                                nc.sync.dma_start(out=dbg2d[:, :, :, :], in_=dbg2b)
                            gt = pg.tile([P, 4, 2, BS], F16, name="gt", tag="ge")
                            for g in range(4):
                                fn = AF.Tanh if g == 2 else AF.Sigmoid
                                nc.scalar.activation(out=gt[:, g, :, :],
                                                     in_=pss[g], func=fn,
                                                     scale=INV_S)
                            cs = cbr[:, 2 * kp:2 * kp + 2, :]
                            if t == 0:
                                nc.vector.tensor_mul(out=cs, in0=gt[:, 0, :, :],
                                                     in1=gt[:, 2, :, :])
                            else:
                                u = pu.tile([P, 2, BS], F32, name="u", tag="u")
                                nc.vector.tensor_mul(out=u, in0=gt[:, 0, :, :],
                                                     in1=gt[:, 2, :, :])
                                nc.vector.tensor_mul(out=cs, in0=gt[:, 1, :, :],
                                                     in1=cs)
                                nc.vector.tensor_add(out=cs, in0=cs, in1=u)
                            nc.scalar.activation(out=gt[:, 2, :, :], in_=cs,
                                                 func=AF.Tanh)
                            xi = 2 * br + kp
                            nc.vector.tensor_mul(out=xe16[xi],
                                                 in0=gt[:, 3, :, :],
                                                 in1=gt[:, 2, :, :])
                    # score/q/r matvecs against the 3 packed columns (clean
                    # fp16 state, 64x: descale folded into exp / A_t / ctxw)
                    for j in range(NJ):
                        psq = psum_q.tile([P, 3], F32, name="psq", tag="psq")
                        for k in range(KD):
                            nc.tensor.matmul(psq,
                                             xe16[k // 2][:, k % 2,
                                                          j * P:(j + 1) * P],
                                             v3t[:, k, :],
                                             start=(k == 0), stop=(k == KD - 1))
                        nc.scalar.copy(out=sqr[:, j, t, :], in_=psq)
                    prev16 = xe16

            # ---------------- decoder attention / ytld --------------------
            if upto == "enc":
                nc.vector.tensor_copy(out=osb, in_=sqr[:, :, 0, 0:1])
                nc.sync.dma_start(out=dbgd.rearrange("(j p) t c -> p j t c", p=P),
                                  in_=sqr)
            for j in range(NJ if upto in ("beta", "ytld", "dec", "full") else 0):
                beta = psm.tile([P, T], F32, name="beta", tag=f"beta{j}")
                ssum = psm.tile([P, 1], F32, name="ssum", tag=f"bsum{j}")
                nc.scalar.activation(out=beta, in_=sqr[:, j, :, 0], func=AF.Exp,
                                     scale=INV_S, accum_out=ssum)
                inv = psm.tile([P, 1], F32, name="inv", tag=f"binv{j}")
                nc.vector.reciprocal(out=inv, in_=ssum)
                tmp9 = psm.tile([P, T], F32, name="tmp9", tag=f"tmp9{j}")
                eq = psm.tile([P, 1], F32, name="eq", tag=f"eq{j}")
                nc.vector.tensor_mul(out=tmp9, in0=beta, in1=sqr[:, j, :, 1])
                nc.vector.reduce_sum(out=eq, in_=tmp9, axis=mybir.AxisListType.X)
                nc.vector.tensor_scalar_mul(out=eq, in0=eq, scalar1=inv)
                nc.vector.tensor_scalar_mul(out=A_t[:, j:j + 1], in0=eq,
                                            scalar1=INV_S)
                tmp9b = psm.tile([P, T], F32, name="tmp9b", tag=f"tmp9b{j}")
                er = psm.tile([P, 1], F32, name="er", tag=f"er{j}")
                nc.vector.tensor_mul(out=tmp9b, in0=beta, in1=sqr[:, j, :, 2])
                nc.vector.reduce_sum(out=er, in_=tmp9b, axis=mybir.AxisListType.X)
                nc.vector.tensor_scalar_mul(out=er, in0=er, scalar1=inv)
                # ctxw = er/S + fcf_b  (bias folded so the tail is one op)
                nc.vector.tensor_scalar_mul(out=ctxw[:, j:j + 1], in0=er,
                                            scalar1=INV_S)
                nc.vector.tensor_scalar_add(out=ctxw[:, j:j + 1],
                                            in0=ctxw[:, j:j + 1],
                                            scalar1=scal["fcf_b"])

            # ytld interleaved with ones, transposed; 64x applied in the copy
            if upto in ("ytld", "dec", "full"):
                nc.vector.memset(ytld2, 1.0)
                for j in range(NJ):
                    nc.vector.tensor_scalar_mul(out=ytld2[:, j, 0:2 * T:2],
                                                in0=yb[:, j, :],
                                                scalar1=scal["w_y"])
                    nc.vector.tensor_scalar_add(out=ytld2[:, j, 0:2 * T:2],
                                                in0=ytld2[:, j, 0:2 * T:2],
                                                scalar1=A_t[:, j:j + 1])
                for j in range(NJ):
                    tpy = psum_t.tile([2 * T, P], F16, name="tpy", tag="pst")
                    nc.tensor.transpose(tpy, ytld2[:, j, :], ident)
                    nc.scalar.activation(out=ytldT[:, j * P:(j + 1) * P],
                                         in_=tpy, func=AF.Copy, scale=S)
                # replicas at 4 row strips per step for the K=2 input quads
                for t in range(T):
                    for g in range(4):
                        nc.sync.dma_start(out=ytA[t][32 * g:32 * g + 2, :],
                                          in_=ytldT[2 * t:2 * t + 2, :])

        if upto in ("beta", "ytld"):
            nc.vector.tensor_copy(out=osb, in_=A_t.unsqueeze(2))

        # keep the PE warm across the attention/ytld gap
        if upto in ("dec", "full"):
            _warm(N_WARM_MID)

        # ---------------- decoder recurrence ----------------
        ndec = T if upto in ("dec", "full") else 0
        dT16p = None
        pdt16 = ctx.enter_context(tc.tile_pool(name="pdt16", bufs=2))
        with tc.tile_pool(name="pgd", bufs=5) as pgd:
            for t in range(ndec):
                dprev16 = dT16p
                dT16p = [pdt16.tile([P, 2, BS], F16, name=f"dT16_{i}",
                                    tag=f"dT16_{i}") for i in range(KD // 2)]
                for kp in range(KD // 2):
                    pss = [psum_g.tile([P, 2, BS], F32, name=f"psd{g}",
                                       tag="psg") for g in range(4)]
                    for half in range(2):
                        for g in range(4):
                            m = g * KD + 2 * kp + half
                            nc.tensor.matmul(
                                pss[g][:, half, :],
                                widR[32 * g:32 * g + 2, m * P:(m + 1) * P],
                                ytA[t][32 * g:32 * g + 2, :],
                                start=True, stop=(t == 0),
                                tile_position=(32 * g, 0))
                        if t > 0:
                            for g in range(4):
                                m = g * KD + 2 * kp + half
                                for k in range(KD):
                                    nc.tensor.matmul(
                                        pss[g][:, half, :],
                                        whhd[:, k, m * P:(m + 1) * P],
                                        dprev16[k // 2][:, k % 2, :],
                                        start=False, stop=(k == KD - 1))
                    gt = pgd.tile([P, 4, 2, BS], F16, name="gtd", tag="gd")
                    for g in range(4):
                        fn = AF.Tanh if g == 2 else AF.Sigmoid
                        nc.scalar.activation(out=gt[:, g, :, :], in_=pss[g],
                                             func=fn, scale=INV_S)
                    cs = cd[:, 2 * kp:2 * kp + 2, :]
                    if t == 0:
                        nc.vector.tensor_mul(out=cs, in0=gt[:, 0, :, :],
                                             in1=gt[:, 2, :, :])
                    else:
                        u = pu.tile([P, 2, BS], F32, name="ud", tag="u")
                        nc.vector.tensor_mul(out=u, in0=gt[:, 0, :, :],
                                             in1=gt[:, 2, :, :])
                        nc.vector.tensor_mul(out=cs, in0=gt[:, 1, :, :], in1=cs)
                        nc.vector.tensor_add(out=cs, in0=cs, in1=u)
                    nc.scalar.activation(out=gt[:, 2, :, :], in_=cs, func=AF.Tanh)
                    nc.vector.tensor_mul(out=dT16p[kp], in0=gt[:, 3, :, :],
                                         in1=gt[:, 2, :, :])

        # ---------------- output ----------------
        if upto == "dec":
            nc.vector.tensor_copy(out=osb, in_=cd[:, 0:NJ, 0:1])
        if upto == "full":
            for j in range(NJ):
                psf = psum_q.tile([P, 1], F32, name="psf", tag="psq")
                for k in range(KD):
                    nc.tensor.matmul(psf,
                                     dT16p[k // 2][:, k % 2, j * P:(j + 1) * P],
                                     wdt[:, k, :],
                                     start=(k == 0), stop=(k == KD - 1))
                nc.vector.scalar_tensor_tensor(
                    out=osb[:, j, :], in0=psf, scalar=INV_S,
                    in1=ctxw[:, j:j + 1], op0=ALU.mult, op1=ALU.add)
        nc.sync.dma_start(out=outd.rearrange("(j p) c -> p j c", p=P), in_=osb)

    nc.compile()
    return nc


def _run(inputs, trace=False, upto="full"):
    weights, scal = _pack_weights(inputs)
    nc = _build(scal, upto=upto)
    X = np.ascontiguousarray(_np(inputs["X"]))
    Y = np.ascontiguousarray(_np(inputs["y_prev"]))
    in_maps = []
    for c in range(NCORES):
        m = dict(weights)
        m["x"] = np.ascontiguousarray(X[c * BS:(c + 1) * BS])
        m["y"] = np.ascontiguousarray(Y[c * BS:(c + 1) * BS])
        in_maps.append(m)
    res = run_bass_kernel_spmd(nc, in_maps, core_ids=list(range(NCORES)), trace=trace)
    out = np.concatenate([np.asarray(res.results[i]["out"]) for i in range(NCORES)],
                         axis=0).astype(np.float32)
    return out, res


def kernel(**inputs):
    out, _ = _run(inputs, trace=False)
    return out
